# revision 1
# baseline (speedup 1.0000x reference)
"""KAN Fourier-linear kernel for 8 Trainium2 NeuronCores.

y[n,o] = sum_{i,g} C0[o,i,g]*cos(g*x[n,i]) + C1[o,i,g]*sin(g*x[n,i]) + bias[o]

Strategy (data-parallel over n, 4096 rows/core):
  - Features F[k, n] for k=(g,trig,i) computed on-chip:
      v   = int32(x*a_g + C_g)            # gpsimd tensor_scalar (round-to-nearest)
      r_g = x - v*(2pi/g)                 # DVE scalar_tensor_tensor (fp32)
      F   = Sin(scale=g, bias=b)(r_g)     # ACT spline, arg in [-5pi/4, 3pi/4]
    cos and sin share one reduced r_g (phase moved into ACT bias).
  - y.T tile = W.T @ F via PE, K=4096 accumulated in PSUM (bf16 inputs).
  - Host: transpose/shard x, reorder fouriercoeffs, assemble y.
"""
import math
import numpy as np
from contextlib import ExitStack

import concourse.bass as bass
import concourse.mybir as mybir
import concourse.tile as tile
from concourse import bacc
from concourse.bass_utils import run_bass_kernel_spmd

import ml_dtypes

N_CORES = 8
N_TOTAL = 32768
N_SHARD = N_TOTAL // N_CORES        # 4096 rows per core
INDIM = 128
OUTDIM = 256
GRID = 16
K_TOT = 2 * GRID * INDIM            # 4096
SP = 2                              # n-superpasses per core
S = N_SHARD // SP                   # 2048 columns per superpass
CH = 512                            # matmul moving chunk
TWO_PI = 2.0 * math.pi

FP32 = mybir.dt.float32
BF16 = mybir.dt.bfloat16
I32 = mybir.dt.int32


def _g_consts(g: int):
    a = np.float32(g / TWO_PI)
    phat = np.float32(TWO_PI / g)
    m = 2.0 ** math.ceil(math.log2(0.960 * g + 0.14))
    c = np.float32(m + 0.125)
    b_s = np.float32(m * g * float(phat))      # == 2pi*m up to fp32, matched to phat
    b_c = np.float32(float(b_s) + math.pi / 2.0)
    return a, phat, c, b_s, b_c


_CACHED = {}


def _build():
    if "nc" in _CACHED:
        return _CACHED["nc"]
    nc = bacc.Bacc("TRN2", target_bir_lowering=False, debug=False,
                   num_devices=N_CORES)
    xt_d = nc.dram_tensor("xt", [INDIM, N_SHARD], FP32, kind="ExternalInput").ap()
    w_d = nc.dram_tensor("w", [INDIM, 32 * OUTDIM], BF16, kind="ExternalInput").ap()
    bt_d = nc.dram_tensor("bt", [INDIM, 32], FP32, kind="ExternalInput").ap()
    bias_d = nc.dram_tensor("bias", [INDIM, 2], FP32, kind="ExternalInput").ap()
    yt_d = nc.dram_tensor("yt", [OUTDIM, N_SHARD], FP32, kind="ExternalOutput").ap()

    with tile.TileContext(nc) as tc, ExitStack() as ctx:
        cpool = ctx.enter_context(tc.tile_pool(name="const", bufs=1))
        vpool = ctx.enter_context(tc.tile_pool(name="v", bufs=2))
        rpool = ctx.enter_context(tc.tile_pool(name="r", bufs=3))
        fpool = ctx.enter_context(tc.tile_pool(name="f", bufs=4))
        ypool = ctx.enter_context(tc.tile_pool(name="y", bufs=2))
        ppool = ctx.enter_context(tc.tile_pool(name="psum", bufs=1, space="PSUM"))

        xt = cpool.tile([INDIM, N_SHARD], FP32)
        nc.sync.dma_start(xt[:], xt_d[:])
        wt = cpool.tile([INDIM, 32 * OUTDIM], BF16)
        nc.sync.dma_start(wt[:], w_d[:])
        bt = cpool.tile([INDIM, 32], FP32)
        nc.sync.dma_start(bt[:], bt_d[:])
        bias = cpool.tile([INDIM, 2], FP32)
        nc.sync.dma_start(bias[:], bias_d[:])

        for sp in range(SP):
            xs = xt[:, sp * S:(sp + 1) * S]
            psum0 = ppool.tile([128, S], FP32, tag="p0")
            psum1 = ppool.tile([128, S], FP32, tag="p1")
            psums = [psum0, psum1]
            for gi in range(GRID):
                g = gi + 1
                a, phat, c, b_s, b_c = _g_consts(g)
                v = vpool.tile([INDIM, S], I32, tag="v")
                nc.gpsimd.tensor_scalar(v[:], xs, float(a), float(c),
                                        mybir.AluOpType.mult, mybir.AluOpType.add)
                r = rpool.tile([INDIM, S], FP32, tag="r")
                nc.vector.scalar_tensor_tensor(r[:], v[:], float(-phat), xs,
                                               mybir.AluOpType.mult,
                                               mybir.AluOpType.add)
                for t in range(2):            # 0=cos, 1=sin
                    kt = 2 * gi + t
                    f = fpool.tile([INDIM, S], BF16, tag="f")
                    nc.scalar.activation(f[:], r[:],
                                         mybir.ActivationFunctionType.Sin,
                                         bias=bt[:, kt:kt + 1], scale=float(g))
                    for oh in range(2):
                        lhsT = wt[:, kt * OUTDIM + oh * 128:
                                  kt * OUTDIM + oh * 128 + 128]
                        for chi in range(S // CH):
                            nc.tensor.matmul(
                                psums[oh][:, chi * CH:(chi + 1) * CH],
                                lhsT, f[:, chi * CH:(chi + 1) * CH],
                                start=(kt == 0), stop=(kt == 31),
                            )
            for oh in range(2):
                y = ypool.tile([128, S], FP32, tag=f"y{oh}")
                nc.vector.tensor_scalar(y[:], psums[oh][:], bias[:, oh:oh + 1],
                                        None, mybir.AluOpType.add)
                nc.sync.dma_start(
                    yt_d[oh * 128:(oh + 1) * 128, sp * S:(sp + 1) * S], y[:])

    nc.compile()
    _CACHED["nc"] = nc
    return nc


def _prep_inputs(x: np.ndarray, fouriercoeffs: np.ndarray, bias: np.ndarray):
    xt = np.ascontiguousarray(x.astype(np.float32, copy=False).T)  # (128, 32768)
    # W2[k, o], k = (g-1)*256 + t*128 + i
    w2 = np.ascontiguousarray(
        fouriercoeffs.astype(np.float32, copy=False).transpose(3, 0, 2, 1)
    ).reshape(K_TOT, OUTDIM)
    w_sb = np.ascontiguousarray(
        w2.reshape(32, 128, OUTDIM).transpose(1, 0, 2).reshape(128, 32 * OUTDIM)
    ).astype(ml_dtypes.bfloat16)
    bvals = np.empty(32, np.float32)
    for gi in range(GRID):
        _, _, _, b_s, b_c = _g_consts(gi + 1)
        bvals[2 * gi] = b_c
        bvals[2 * gi + 1] = b_s
    bt = np.tile(bvals[None, :], (INDIM, 1)).astype(np.float32)
    bias_sb = np.ascontiguousarray(
        bias.reshape(2, 128).T.astype(np.float32))      # (128, 2)
    return xt, w_sb, bt, bias_sb


def kernel(x: np.ndarray, fouriercoeffs: np.ndarray, bias: np.ndarray,
           _trace: bool = False):
    x = np.asarray(x)
    fouriercoeffs = np.asarray(fouriercoeffs)
    bias = np.asarray(bias)
    orig_shape = x.shape
    x2 = x.reshape(-1, INDIM)
    assert x2.shape == (N_TOTAL, INDIM), x2.shape

    nc = _build()
    xt, w_sb, bt, bias_sb = _prep_inputs(x2, fouriercoeffs, bias)
    in_maps = []
    for c in range(N_CORES):
        in_maps.append({
            "xt": np.ascontiguousarray(xt[:, c * N_SHARD:(c + 1) * N_SHARD]),
            "w": w_sb,
            "bt": bt,
            "bias": bias_sb,
        })
    res = run_bass_kernel_spmd(nc, in_maps, list(range(N_CORES)),
                               trace=_trace)
    yt = np.concatenate([res.results[c]["yt"] for c in range(N_CORES)], axis=1)
    y = np.ascontiguousarray(yt.T).astype(np.float32)
    if _trace:
        kernel._last_result = res
    return y.reshape(*orig_shape[:-1], OUTDIM)



# revision 3
# speedup vs baseline: 1.1828x; 1.1828x over previous
"""KAN Fourier-linear kernel for 8 Trainium2 NeuronCores.

y[n,o] = sum_{i,g} C0[o,i,g]*cos(g*x[n,i]) + C1[o,i,g]*sin(g*x[n,i]) + bias[o]

Strategy (data-parallel over n, 4096 rows/core):
  - Base features cos/sin(g*x) for g in {1,2,3,4,8} via range-reduce + ACT Sin:
      v   = int32(x*a_g + C_g)            # gpsimd tensor_scalar
      r_g = x - v*(2pi/g)                 # DVE scalar_tensor_tensor (fp32)
      f   = Sin(scale=g, bias=b)(r_g)     # ACT spline, arg in [-5pi/4, 3pi/4]
  - Remaining 22 features as single DVE bf16 multiplies using
      sin(a)*2cos(b) = sin(a+b) + sin(a-b),  cos(a)*2cos(b) = cos(a+b) + cos(a-b)
    with the correction terms folded into the weights on the host
    (W_comp = M^-T W_true for the 32x32 expansion matrix M).
  - y.T tile = W.T @ F via PE, K=4096 accumulated in PSUM (bf16 inputs).
  - 4 superpasses of 1024 cols with alternating PSUM bank sets; chunked DMAs.
"""
import math
import numpy as np
from contextlib import ExitStack

import concourse.bass as bass
import concourse.mybir as mybir
import concourse.tile as tile
from concourse import bacc
from concourse.bass_utils import run_bass_kernel_spmd

import ml_dtypes

N_CORES = 8
N_TOTAL = 32768
N_SHARD = N_TOTAL // N_CORES        # 4096 rows per core
INDIM = 128
OUTDIM = 256
GRID = 16
K_TOT = 32                          # computed feature tiles
SP = 4                              # n-superpasses per core
S = N_SHARD // SP                   # 1024 columns per superpass
CH = 512                            # matmul moving chunk
TWO_PI = 2.0 * math.pi

FP32 = mybir.dt.float32
BF16 = mybir.dt.bfloat16
I32 = mybir.dt.int32

BASE = [1, 2, 3, 4, 8]
# (tile name, source tile, multiplier b)  -- tile = src * (2*cos(b*x))
LEAVES = [
    ("Lc5", "c1", 4), ("Ls5", "s1", 4),
    ("Lc6", "c2", 4), ("Ls6", "s2", 4),
    ("Lc7", "c3", 4), ("Ls7", "s3", 4),
    ("Lc9", "c1", 8), ("Ls9", "s1", 8),
    ("Lc10", "c2", 8), ("Ls10", "s2", 8),
    ("Lc11", "c3", 8), ("Ls11", "s3", 8),
    ("Lc12", "c4", 8), ("Ls12", "s4", 8),
    ("Lc16", "c8", 8), ("Ls16", "s8", 8),
    ("Lc13", "Lc5", 8), ("Ls13", "Ls5", 8),
    ("Lc14", "Lc6", 8), ("Ls14", "Ls6", 8),
    ("Lc15", "Lc7", 8), ("Ls15", "Ls7", 8),
]
KT_ORDER = (["c1", "s1", "c2", "s2", "c3", "s3", "c4", "s4", "c8", "s8"]
            + [l[0] for l in LEAVES])
# leaf tiles that feed further products need double buffering across superpasses
LEAF_SRC = {"Lc5", "Ls5", "Lc6", "Ls6", "Lc7", "Ls7"}


def _g_consts(g: int):
    a = np.float32(g / TWO_PI)
    phat = np.float32(TWO_PI / g)
    m = 2.0 ** math.ceil(math.log2(0.960 * g + 0.14))
    c = np.float32(m + 0.125)
    b_s = np.float32(m * g * float(phat))      # == 2pi*m up to fp32, matched to phat
    b_c = np.float32(float(b_s) + math.pi / 2.0)
    return a, phat, c, b_s, b_c


def _expansion_matrix():
    """M[kt, j] with f_comp = M @ f_true + m0; true feature j = 2(g-1)+{0:cos,1:sin}."""
    def expand_mult(expA, b):
        out = {}

        def add(k, v):
            out[k] = out.get(k, 0.0) + v

        for k, coef in expA.items():
            if k == "const":
                add(("c", b), 2.0 * coef)
                continue
            t, g = k
            hi, lo = g + b, g - b
            if t == "c":
                add(("c", hi), coef)
                if lo == 0:
                    add("const", coef)
                else:
                    add(("c", abs(lo)), coef)
            else:
                add(("s", hi), coef)
                if lo != 0:
                    add(("s", abs(lo)), coef if lo > 0 else -coef)
        return out

    exps = {}
    for g in BASE:
        exps[f"c{g}"] = {("c", g): 1.0}
        exps[f"s{g}"] = {("s", g): 1.0}
    for name, src, b in LEAVES:
        exps[name] = expand_mult(exps[src], b)

    M = np.zeros((32, 32))
    m0 = np.zeros(32)
    for kt, key in enumerate(KT_ORDER):
        for k, coef in exps[key].items():
            if k == "const":
                m0[kt] = coef
            else:
                t, g = k
                M[kt, 2 * (g - 1) + (0 if t == "c" else 1)] = coef
    return M, m0


_CACHED = {}


def _build():
    if "nc" in _CACHED:
        return _CACHED["nc"]
    nc = bacc.Bacc("TRN2", target_bir_lowering=False, debug=False,
                   num_devices=N_CORES)
    xt_d = nc.dram_tensor("xt", [INDIM, N_SHARD], FP32, kind="ExternalInput").ap()
    w_d = nc.dram_tensor("w", [INDIM, K_TOT * OUTDIM], BF16, kind="ExternalInput").ap()
    bt_d = nc.dram_tensor("bt", [INDIM, 2 * len(BASE)], FP32, kind="ExternalInput").ap()
    bias_d = nc.dram_tensor("bias", [INDIM, 2], FP32, kind="ExternalInput").ap()
    yt_d = nc.dram_tensor("yt", [OUTDIM, N_SHARD], FP32, kind="ExternalOutput").ap()

    with tile.TileContext(nc) as tc, ExitStack() as ctx:
        cpool = ctx.enter_context(tc.tile_pool(name="const", bufs=1))
        vpool = ctx.enter_context(tc.tile_pool(name="v", bufs=2))
        rpool = ctx.enter_context(tc.tile_pool(name="r", bufs=3))
        bpool = ctx.enter_context(tc.tile_pool(name="base", bufs=2))
        spool = ctx.enter_context(tc.tile_pool(name="leafsrc", bufs=2))
        lpool = ctx.enter_context(tc.tile_pool(name="leaf", bufs=1))
        ypool = ctx.enter_context(tc.tile_pool(name="y", bufs=2))
        ppool = ctx.enter_context(tc.tile_pool(name="psum", bufs=2, space="PSUM"))

        xt = cpool.tile([INDIM, N_SHARD], FP32)
        wt = cpool.tile([INDIM, K_TOT * OUTDIM], BF16)
        bt = cpool.tile([INDIM, 2 * len(BASE)], FP32)
        bias = cpool.tile([INDIM, 2], FP32)
        nc.sync.dma_start(bt[:], bt_d[:])
        nc.sync.dma_start(bias[:], bias_d[:])
        nc.sync.dma_start(wt[:, 0:2048], w_d[:, 0:2048])
        nc.sync.dma_start(xt[:, 0:S], xt_d[:, 0:S])
        for j in range(1, 4):
            nc.sync.dma_start(wt[:, j * 2048:(j + 1) * 2048],
                              w_d[:, j * 2048:(j + 1) * 2048])
        for sp in range(1, SP):
            nc.sync.dma_start(xt[:, sp * S:(sp + 1) * S],
                              xt_d[:, sp * S:(sp + 1) * S])

        psums_of = {}

        def emit_feats(sp):
            xs = xt[:, sp * S:(sp + 1) * S]
            tiles = {}
            for g in BASE:
                a, phat, c, b_s, b_c = _g_consts(g)
                v = vpool.tile([INDIM, S], I32, tag="v")
                nc.gpsimd.tensor_scalar(v[:], xs, float(a), float(c),
                                        mybir.AluOpType.mult, mybir.AluOpType.add)
                r = rpool.tile([INDIM, S], FP32, tag="r")
                nc.vector.scalar_tensor_tensor(r[:], v[:], float(-phat), xs,
                                               mybir.AluOpType.mult,
                                               mybir.AluOpType.add)
                gi = BASE.index(g)
                for t, col in (("c", 2 * gi), ("s", 2 * gi + 1)):
                    f = bpool.tile([INDIM, S], BF16, tag=f"{t}{g}")
                    nc.scalar.activation(f[:], r[:],
                                         mybir.ActivationFunctionType.Sin,
                                         bias=bt[:, col:col + 1], scale=float(g))
                    tiles[f"{t}{g}"] = f
            for b in (4, 8):
                cp = bpool.tile([INDIM, S], BF16, tag=f"Cp{b}")
                nc.vector.tensor_scalar_mul(cp[:], tiles[f"c{b}"][:], 2.0)
                tiles[f"Cp{b}"] = cp
            for name, src, b in LEAVES:
                pool = spool if name in LEAF_SRC else lpool
                f = pool.tile([INDIM, S], BF16, tag=name)
                nc.vector.tensor_mul(f[:], tiles[src][:], tiles[f"Cp{b}"][:])
                tiles[name] = f
            return tiles

        def emit_mms(sp, tiles):
            psums = [ppool.tile([128, S], FP32, tag=f"p{oh}", name=f"psum{oh}")
                     for oh in range(2)]
            psums_of[sp] = psums
            for kt, key in enumerate(KT_ORDER):
                f = tiles[key]
                for oh in range(2):
                    lhsT = wt[:, kt * OUTDIM + oh * 128:
                              kt * OUTDIM + oh * 128 + 128]
                    for chi in range(S // CH):
                        nc.tensor.matmul(
                            psums[oh][:, chi * CH:(chi + 1) * CH],
                            lhsT, f[:, chi * CH:(chi + 1) * CH],
                            start=(kt == 0), stop=(kt == 31),
                        )

        def emit_out(sp):
            for oh in range(2):
                y = ypool.tile([128, S], FP32, tag=f"y{oh}")
                nc.scalar.activation(y[:], psums_of[sp][oh][:],
                                     mybir.ActivationFunctionType.Identity,
                                     bias=bias[:, oh:oh + 1])
                nc.sync.dma_start(
                    yt_d[oh * 128:(oh + 1) * 128, sp * S:(sp + 1) * S], y[:])

        tiles = emit_feats(0)
        emit_mms(0, tiles)
        for sp in range(1, SP):
            tiles = emit_feats(sp)
            emit_out(sp - 1)
            emit_mms(sp, tiles)
        emit_out(SP - 1)

    nc.compile()
    _CACHED["nc"] = nc
    return nc


def _prep_inputs(x: np.ndarray, fouriercoeffs: np.ndarray, bias: np.ndarray):
    xt = np.ascontiguousarray(x.astype(np.float32, copy=False).T)  # (128, 32768)

    M, m0 = _expansion_matrix()
    fc = fouriercoeffs.astype(np.float64, copy=False)
    W_true = np.zeros((OUTDIM, INDIM, 32))
    for g in range(1, GRID + 1):
        W_true[:, :, 2 * (g - 1)] = fc[0, :, :, g - 1]
        W_true[:, :, 2 * (g - 1) + 1] = fc[1, :, :, g - 1]
    W_comp = np.linalg.solve(M.T, W_true.reshape(-1, 32).T).T.reshape(
        OUTDIM, INDIM, 32)
    # w_sb[i, kt*256 + o] = W_comp[o, i, kt]
    w_sb = np.ascontiguousarray(
        W_comp.transpose(1, 2, 0).reshape(INDIM, 32 * OUTDIM)
    ).astype(ml_dtypes.bfloat16)

    bias_new = bias.reshape(-1).astype(np.float64).copy()
    kt16 = KT_ORDER.index("Lc16")
    bias_new -= W_comp[:, :, kt16].sum(axis=1) * m0[kt16]

    bvals = np.empty(2 * len(BASE), np.float32)
    for gi, g in enumerate(BASE):
        _, _, _, b_s, b_c = _g_consts(g)
        bvals[2 * gi] = b_c
        bvals[2 * gi + 1] = b_s
    bt = np.tile(bvals[None, :], (INDIM, 1)).astype(np.float32)
    bias_sb = np.ascontiguousarray(
        bias_new.reshape(2, 128).T.astype(np.float32))      # (128, 2)
    return xt, w_sb, bt, bias_sb


def kernel(x: np.ndarray, fouriercoeffs: np.ndarray, bias: np.ndarray,
           _trace: bool = False):
    x = np.asarray(x)
    fouriercoeffs = np.asarray(fouriercoeffs)
    bias = np.asarray(bias)
    orig_shape = x.shape
    x2 = x.reshape(-1, INDIM)
    assert x2.shape == (N_TOTAL, INDIM), x2.shape

    nc = _build()
    xt, w_sb, bt, bias_sb = _prep_inputs(x2, fouriercoeffs, bias)
    in_maps = []
    for c in range(N_CORES):
        in_maps.append({
            "xt": np.ascontiguousarray(xt[:, c * N_SHARD:(c + 1) * N_SHARD]),
            "w": w_sb,
            "bt": bt,
            "bias": bias_sb,
        })
    res = run_bass_kernel_spmd(nc, in_maps, list(range(N_CORES)),
                               trace=_trace)
    yt = np.concatenate([res.results[c]["yt"] for c in range(N_CORES)], axis=1)
    y = np.ascontiguousarray(yt.T).astype(np.float32)
    if _trace:
        kernel._last_result = res
    return y.reshape(*orig_shape[:-1], OUTDIM)


# revision 6
# speedup vs baseline: 1.2200x; 1.0315x over previous
"""KAN Fourier-linear kernel for 8 Trainium2 NeuronCores.

y[n,o] = sum_{i,g} C0[o,i,g]*cos(g*x[n,i]) + C1[o,i,g]*sin(g*x[n,i]) + bias[o]

Strategy (data-parallel over n, 4096 rows/core):
  - Base features cos/sin(g*x) for g in {1,2,3,4,8} via range-reduce + ACT Sin:
      v   = int32(x*a_g + C_g)            # gpsimd tensor_scalar
      r_g = x - v*(2pi/g)                 # DVE scalar_tensor_tensor (fp32)
      f   = Sin(scale=g, bias=b)(r_g)     # ACT spline, arg in [-5pi/4, 3pi/4]
  - Remaining 22 features as single DVE bf16 multiplies using
      sin(a)*2cos(b) = sin(a+b) + sin(a-b),  cos(a)*2cos(b) = cos(a+b) + cos(a-b)
    with the correction terms folded into the weights on the host
    (W_comp = M^-T W_true for the 32x32 expansion matrix M).
  - y.T tile = W.T @ F via PE, K=4096 accumulated in PSUM (bf16 inputs).
  - 4 superpasses of 1024 cols with alternating PSUM bank sets; chunked DMAs.
"""
import math
import numpy as np
from contextlib import ExitStack

import concourse.bass as bass
import concourse.mybir as mybir
import concourse.tile as tile
from concourse import bacc
from concourse.bass_utils import run_bass_kernel_spmd

import ml_dtypes

N_CORES = 8
N_TOTAL = 32768
N_SHARD = N_TOTAL // N_CORES        # 4096 rows per core
INDIM = 128
OUTDIM = 256
GRID = 16
K_TOT = 32                          # computed feature tiles
SP = 4                              # n-superpasses per core
S = N_SHARD // SP                   # 1024 columns per superpass
CH = 512                            # matmul moving chunk
TWO_PI = 2.0 * math.pi

FP32 = mybir.dt.float32
BF16 = mybir.dt.bfloat16
I32 = mybir.dt.int32

BASE = [1, 2, 3, 4, 8]
# (tile name, source tile, multiplier b)  -- tile = src * (2*cos(b*x))
LEAVES = [
    ("Lc5", "c1", 4), ("Ls5", "s1", 4),
    ("Lc6", "c2", 4), ("Ls6", "s2", 4),
    ("Lc7", "c3", 4), ("Ls7", "s3", 4),
    ("Lc9", "c1", 8), ("Ls9", "s1", 8),
    ("Lc10", "c2", 8), ("Ls10", "s2", 8),
    ("Lc11", "c3", 8), ("Ls11", "s3", 8),
    ("Lc12", "c4", 8), ("Ls12", "s4", 8),
    ("Lc16", "c8", 8), ("Ls16", "s8", 8),
    ("Lc13", "Lc5", 8), ("Ls13", "Ls5", 8),
    ("Lc14", "Lc6", 8), ("Ls14", "Ls6", 8),
    ("Lc15", "Lc7", 8), ("Ls15", "Ls7", 8),
]
KT_ORDER = (["c1", "s1", "c2", "s2", "c3", "s3", "c4", "s4", "c8", "s8"]
            + [l[0] for l in LEAVES])
# leaf tiles that feed further products need double buffering across superpasses
LEAF_SRC = {"Lc5", "Ls5", "Lc6", "Ls6", "Lc7", "Ls7"}


def _g_consts(g: int):
    a = np.float32(g / TWO_PI)
    phat = np.float32(TWO_PI / g)
    m = 2.0 ** math.ceil(math.log2(0.960 * g + 0.14))
    c = np.float32(m + 0.125)
    b_s = np.float32(m * g * float(phat))      # == 2pi*m up to fp32, matched to phat
    b_c = np.float32(float(b_s) + math.pi / 2.0)
    return a, phat, c, b_s, b_c


def _expansion_matrix():
    """M[kt, j] with f_comp = M @ f_true + m0; true feature j = 2(g-1)+{0:cos,1:sin}."""
    def expand_mult(expA, b):
        out = {}

        def add(k, v):
            out[k] = out.get(k, 0.0) + v

        for k, coef in expA.items():
            if k == "const":
                add(("c", b), 2.0 * coef)
                continue
            t, g = k
            hi, lo = g + b, g - b
            if t == "c":
                add(("c", hi), coef)
                if lo == 0:
                    add("const", coef)
                else:
                    add(("c", abs(lo)), coef)
            else:
                add(("s", hi), coef)
                if lo != 0:
                    add(("s", abs(lo)), coef if lo > 0 else -coef)
        return out

    exps = {}
    for g in BASE:
        exps[f"c{g}"] = {("c", g): 1.0}
        exps[f"s{g}"] = {("s", g): 1.0}
    for name, src, b in LEAVES:
        exps[name] = expand_mult(exps[src], b)

    M = np.zeros((32, 32))
    m0 = np.zeros(32)
    for kt, key in enumerate(KT_ORDER):
        for k, coef in exps[key].items():
            if k == "const":
                m0[kt] = coef
            else:
                t, g = k
                M[kt, 2 * (g - 1) + (0 if t == "c" else 1)] = coef
    return M, m0


_CACHED = {}


def _build():
    if "nc" in _CACHED:
        return _CACHED["nc"]
    nc = bacc.Bacc("TRN2", target_bir_lowering=False, debug=False,
                   num_devices=N_CORES)
    xt_d = nc.dram_tensor("xt", [INDIM, N_SHARD], FP32, kind="ExternalInput").ap()
    w_d = nc.dram_tensor("w", [INDIM, K_TOT * OUTDIM], BF16, kind="ExternalInput").ap()
    bt_d = nc.dram_tensor("bt", [INDIM, 2 * len(BASE)], FP32, kind="ExternalInput").ap()
    bias_d = nc.dram_tensor("bias", [INDIM, 2], FP32, kind="ExternalInput").ap()
    yt_d = nc.dram_tensor("yt", [OUTDIM, N_SHARD], FP32, kind="ExternalOutput").ap()

    with tile.TileContext(nc) as tc, ExitStack() as ctx:
        cpool = ctx.enter_context(tc.tile_pool(name="const", bufs=1))
        vpool = ctx.enter_context(tc.tile_pool(name="v", bufs=2))
        rpool = ctx.enter_context(tc.tile_pool(name="r", bufs=3))
        bpool = ctx.enter_context(tc.tile_pool(name="base", bufs=2))
        spool = ctx.enter_context(tc.tile_pool(name="leafsrc", bufs=2))
        lpool = ctx.enter_context(tc.tile_pool(name="leaf", bufs=1))
        ypool = ctx.enter_context(tc.tile_pool(name="y", bufs=2))
        ppool = ctx.enter_context(tc.tile_pool(name="psum", bufs=2, space="PSUM"))

        xt = cpool.tile([INDIM, N_SHARD], FP32)
        wt = cpool.tile([INDIM, K_TOT * OUTDIM], BF16)
        bt = cpool.tile([INDIM, 2 * len(BASE)], FP32)
        bias = cpool.tile([INDIM, 2], FP32)
        nc.sync.dma_start(xt[:, 0:S], xt_d[:, 0:S])
        nc.sync.dma_start(wt[:, 0:2048], w_d[:, 0:2048])
        nc.sync.dma_start(bt[:], bt_d[:])
        nc.sync.dma_start(bias[:], bias_d[:])
        for j in range(1, 4):
            nc.sync.dma_start(xt[:, j * S:(j + 1) * S],
                              xt_d[:, j * S:(j + 1) * S])
            nc.sync.dma_start(wt[:, j * 2048:(j + 1) * 2048],
                              w_d[:, j * 2048:(j + 1) * 2048])

        # PE warm-up: HAM un-throttles after ~3.4us of sustained activity.
        # Burn dummy matmuls on a zeroed scratch tile while input DMAs land
        # so the real matmul stream starts at 2.4 GHz.
        scratch = cpool.tile([128, CH], BF16)
        nc.vector.memset(scratch[:], 0)
        pwarm = ppool.tile([128, CH], FP32, tag="p00", name="pwarm")
        for _ in range(10):
            nc.tensor.matmul(pwarm[:], scratch[:, 0:128], scratch[:],
                             start=True, stop=True)

        psums_of = {}

        def emit_feats(sp):
            xs = xt[:, sp * S:(sp + 1) * S]
            tiles = {}
            for g in BASE:
                a, phat, c, b_s, b_c = _g_consts(g)
                v = vpool.tile([INDIM, S], I32, tag="v")
                nc.gpsimd.tensor_scalar(v[:], xs, float(a), float(c),
                                        mybir.AluOpType.mult, mybir.AluOpType.add)
                r = rpool.tile([INDIM, S], FP32, tag="r")
                nc.vector.scalar_tensor_tensor(r[:], v[:], float(-phat), xs,
                                               mybir.AluOpType.mult,
                                               mybir.AluOpType.add)
                gi = BASE.index(g)
                for t, col in (("c", 2 * gi), ("s", 2 * gi + 1)):
                    f = bpool.tile([INDIM, S], BF16, tag=f"{t}{g}")
                    nc.scalar.activation(f[:], r[:],
                                         mybir.ActivationFunctionType.Sin,
                                         bias=bt[:, col:col + 1], scale=float(g))
                    tiles[f"{t}{g}"] = f
            for b in (4, 8):
                cp = bpool.tile([INDIM, S], BF16, tag=f"Cp{b}")
                nc.vector.tensor_scalar_mul(cp[:], tiles[f"c{b}"][:], 2.0)
                tiles[f"Cp{b}"] = cp
            for name, src, b in LEAVES:
                pool = spool if name in LEAF_SRC else lpool
                f = pool.tile([INDIM, S], BF16, tag=name)
                nc.vector.tensor_mul(f[:], tiles[src][:], tiles[f"Cp{b}"][:])
                tiles[name] = f
            return tiles

        def emit_mms(sp, tiles):
            psums = {}
            for oh in range(2):
                for chi in range(S // CH):
                    psums[(oh, chi)] = ppool.tile(
                        [128, CH], FP32, tag=f"p{oh}{chi}",
                        name=f"psum{oh}{chi}")
            psums_of[sp] = psums
            for kt, key in enumerate(KT_ORDER):
                f = tiles[key]
                for oh in range(2):
                    lhsT = wt[:, kt * OUTDIM + oh * 128:
                              kt * OUTDIM + oh * 128 + 128]
                    for chi in range(S // CH):
                        nc.tensor.matmul(
                            psums[(oh, chi)][:],
                            lhsT, f[:, chi * CH:(chi + 1) * CH],
                            start=(kt == 0), stop=(kt == 31),
                        )

        def emit_out(sp):
            for oh in range(2):
                for chi in range(S // CH):
                    y = ypool.tile([128, CH], FP32, tag=f"y{oh}{chi}",
                                   name=f"y{oh}{chi}")
                    nc.scalar.activation(y[:], psums_of[sp][(oh, chi)][:],
                                         mybir.ActivationFunctionType.Identity,
                                         bias=bias[:, oh:oh + 1])
                    nc.sync.dma_start(
                        yt_d[oh * 128:(oh + 1) * 128,
                             sp * S + chi * CH:sp * S + (chi + 1) * CH], y[:])

        tiles = emit_feats(0)
        emit_mms(0, tiles)
        for sp in range(1, SP):
            tiles = emit_feats(sp)
            emit_out(sp - 1)
            emit_mms(sp, tiles)
        emit_out(SP - 1)

    nc.compile()
    _CACHED["nc"] = nc
    return nc


def _prep_inputs(x: np.ndarray, fouriercoeffs: np.ndarray, bias: np.ndarray):
    xt = np.ascontiguousarray(x.astype(np.float32, copy=False).T)  # (128, 32768)

    M, m0 = _expansion_matrix()
    fc = fouriercoeffs.astype(np.float64, copy=False)
    W_true = np.zeros((OUTDIM, INDIM, 32))
    for g in range(1, GRID + 1):
        W_true[:, :, 2 * (g - 1)] = fc[0, :, :, g - 1]
        W_true[:, :, 2 * (g - 1) + 1] = fc[1, :, :, g - 1]
    W_comp = np.linalg.solve(M.T, W_true.reshape(-1, 32).T).T.reshape(
        OUTDIM, INDIM, 32)
    # w_sb[i, kt*256 + o] = W_comp[o, i, kt]
    w_sb = np.ascontiguousarray(
        W_comp.transpose(1, 2, 0).reshape(INDIM, 32 * OUTDIM)
    ).astype(ml_dtypes.bfloat16)

    bias_new = bias.reshape(-1).astype(np.float64).copy()
    kt16 = KT_ORDER.index("Lc16")
    bias_new -= W_comp[:, :, kt16].sum(axis=1) * m0[kt16]

    bvals = np.empty(2 * len(BASE), np.float32)
    for gi, g in enumerate(BASE):
        _, _, _, b_s, b_c = _g_consts(g)
        bvals[2 * gi] = b_c
        bvals[2 * gi + 1] = b_s
    bt = np.tile(bvals[None, :], (INDIM, 1)).astype(np.float32)
    bias_sb = np.ascontiguousarray(
        bias_new.reshape(2, 128).T.astype(np.float32))      # (128, 2)
    return xt, w_sb, bt, bias_sb


def kernel(x: np.ndarray, fouriercoeffs: np.ndarray, bias: np.ndarray,
           _trace: bool = False):
    x = np.asarray(x)
    fouriercoeffs = np.asarray(fouriercoeffs)
    bias = np.asarray(bias)
    orig_shape = x.shape
    x2 = x.reshape(-1, INDIM)
    assert x2.shape == (N_TOTAL, INDIM), x2.shape

    nc = _build()
    xt, w_sb, bt, bias_sb = _prep_inputs(x2, fouriercoeffs, bias)
    in_maps = []
    for c in range(N_CORES):
        in_maps.append({
            "xt": np.ascontiguousarray(xt[:, c * N_SHARD:(c + 1) * N_SHARD]),
            "w": w_sb,
            "bt": bt,
            "bias": bias_sb,
        })
    res = run_bass_kernel_spmd(nc, in_maps, list(range(N_CORES)),
                               trace=_trace)
    yt = np.concatenate([res.results[c]["yt"] for c in range(N_CORES)], axis=1)
    y = np.ascontiguousarray(yt.T).astype(np.float32)
    if _trace:
        kernel._last_result = res
    return y.reshape(*orig_shape[:-1], OUTDIM)
